# revision 1
# baseline (speedup 1.0000x reference)
"""BitLinear (BitNet a4.8-style) Trainium2 kernel.

Computes  out = act_quant_int4(x) @ ste_ternary(w).T  for
x:[8192,4096] f32, w:[4096,4096] f32, on 8 NeuronCores.

Math structure exploited:
  - act_quant_int4(x) rows are  k/s_t  with integer k in [-7,7],
    s_t = 7/amax_t  (per-token).  The clip to [-8,7] is a no-op since
    |x*s| <= 7 by construction.
  - ste_ternary(w) = q * scale with q in {-1,0,1},
    scale = max(mean|w|, 1e-8)  (global scalar).
  - So out[t,o] = (scale * amax_t / 7) * sum_i k[t,i] * q[o,i].
    The inner sum is an exact small-integer dot product: we run it on the
    PE array in fp8 (e4m3 holds -8..8 and -1..1 exactly; DoubleRow fp8
    accumulates exactly in fp32 PSUM), then scale rows by
    f_t = scale*amax_t/7 during PSUM eviction.

Three launches on 8 cores:
  1. scale pass: per-core partial |w| sums over a 1/8 row shard of wT,
     reduced in 128-element chunks; host finishes the reduction in f64
     and forms the global ternary scale.
  2. w-quant pass: each core ternarizes a 1/8 row shard of wT into fp8
     {-1,0,+1}; host gathers the full quantized wT (16.7 MB).
  3. main pass, data-parallel over tokens x8: each core takes
     x[1024,4096] f32 + the full pre-quantized wT[4096,4096] fp8, int4-
     quantizes x on the fly (abs-max reduce -> scale -> round via the
     +-1.5*2^23 magic trick), PE-transposes the int8 activations, and
     runs DoubleRow fp8 matmuls, scaling rows during PSUM eviction.

w is transposed on the host once (input marshalling) so the contraction
dim lands on SBUF partitions for both operands.
"""

import numpy as np
from contextlib import ExitStack

import concourse.bacc as bacc
import concourse.bass as bass
import concourse.mybir as mybir
import concourse.tile as tile
from concourse.bass_utils import run_bass_kernel_spmd

F32 = mybir.dt.float32
FP8 = mybir.dt.float8e4
ALU = mybir.AluOpType
ACTF = mybir.ActivationFunctionType

TOK, DIN, DOUT = 8192, 4096, 4096
NCORES = 8
TG, OG = 8, 1            # token shards x out-feature shards (data parallel)
TSH = TOK // TG          # 1024 tokens per core
OSH = DOUT // OG         # 4096 out features per core
KSUB = DIN // 128        # 32 contraction sub-tiles
NT = TSH // 128          # 8 token tiles per core
NKQ = 8                  # w held in 8 chunks of 4 ksubs (pipelining)
WSEG = DIN // NCORES     # 512 wT rows per core in launches 1/2
MAGIC = 12582912.0       # 1.5*2^23: float add/sub round-to-nearest-int trick
CLAMP = float(np.nextafter(np.float32(1.5), np.float32(0.0)))
EPS = 1e-8

_CACHE = {}


def _build_scale_nc():
    """Launch 1: per-core partial |w| sums, in 128-element chunks so the
    fp32 accumulation error stays ~1e-7 relative (host finishes in f64)."""
    nc = bacc.Bacc("TRN2", target_bir_lowering=False, debug=False,
                   num_devices=NCORES)
    wseg = nc.dram_tensor("wseg", [WSEG, DIN], F32,
                          kind="ExternalInput").ap()
    psums = nc.dram_tensor("psums", [128, 128], F32,
                           kind="ExternalOutput").ap()
    with tile.TileContext(nc) as tc, ExitStack() as ctx:
        pool = ctx.enter_context(tc.tile_pool(name="w", bufs=4))
        spool = ctx.enter_context(tc.tile_pool(name="s", bufs=1))
        sums = spool.tile([128, 8, 16], F32)
        for i in range(8):
            r0, c0 = (i // 2) * 128, (i % 2) * 2048
            wt = pool.tile([128, 16, 128], F32)
            nc.sync.dma_start(
                out=wt,
                in_=wseg[r0:r0 + 128, c0:c0 + 2048].rearrange(
                    "p (a b) -> p a b", a=16))
            nc.vector.tensor_reduce(
                out=sums[:, i, :], in_=wt, axis=mybir.AxisListType.X,
                op=ALU.add, apply_absolute_value=True)
        nc.sync.dma_start(out=psums, in_=sums.rearrange("p a b -> p (a b)"))
    nc.compile()
    return nc


def _build_wquant_nc():
    """Launch 2: ternarize a [512, 4096] row shard of wT into fp8.
    round(clip(y,-1,1)) == round(clamp(y, +-CLAMP)) for |y|<=2.1."""
    nc = bacc.Bacc("TRN2", target_bir_lowering=False, debug=False,
                   num_devices=NCORES)
    wseg = nc.dram_tensor("wseg", [WSEG, DIN], F32,
                          kind="ExternalInput").ap()
    sca = nc.dram_tensor("sca", [128, 2], F32, kind="ExternalInput").ap()
    wq8 = nc.dram_tensor("wq8", [WSEG, DIN], FP8,
                         kind="ExternalOutput").ap()
    with tile.TileContext(nc) as tc, ExitStack() as ctx:
        const = ctx.enter_context(tc.tile_pool(name="const", bufs=1))
        pool = ctx.enter_context(tc.tile_pool(name="w", bufs=4))
        qpool = ctx.enter_context(tc.tile_pool(name="q", bufs=4))
        scat = const.tile([128, 2], F32)
        nc.sync.dma_start(out=scat, in_=sca)
        # 8 half-tiles, p2 alternating Pool/DVE, for a tight 4-stage pipe.
        for i in range(8):
            r0, c0 = (i // 2) * 128, (i % 2) * 2048
            wt = pool.tile([128, 2048], F32, tag="wt")
            nc.sync.dma_start(
                out=wt, in_=wseg[r0:r0 + 128, c0:c0 + 2048])
            nc.vector.tensor_scalar(
                out=wt, in0=wt, scalar1=scat[:, 0:1], scalar2=CLAMP,
                op0=ALU.mult, op1=ALU.min)
            eng = nc.gpsimd if i % 2 == 0 else nc.vector
            eng.tensor_scalar(
                out=wt, in0=wt, scalar1=-CLAMP, scalar2=MAGIC,
                op0=ALU.max, op1=ALU.add)
            qt = qpool.tile([128, 2048], FP8, tag="qt")
            nc.scalar.activation(out=qt, in_=wt, func=ACTF.Copy,
                                 bias=-MAGIC, scale=1.0)
            nc.sync.dma_start(out=wq8[r0:r0 + 128, c0:c0 + 2048], in_=qt)
    nc.compile()
    return nc


def _build_main_nc(repeat=1):
    nc = bacc.Bacc("TRN2", target_bir_lowering=False, debug=False,
                   num_devices=NCORES)
    xs = nc.dram_tensor("xs", [TSH, DIN], F32, kind="ExternalInput").ap()
    # Pre-quantized w in pair-interleaved layout: wts8[p, s, b, o] is
    # q_{o,i} for i = s*256 + 2p + b.  This matches what the fp8-pair
    # (uint16) xbar DMA transpose produces for the activations, so the
    # contraction index mapping agrees between lhsT and rhs.
    wts8 = nc.dram_tensor("wts8", [128, 16, 2, OSH], FP8,
                          kind="ExternalInput").ap()
    sca = nc.dram_tensor("sca", [128, 2], F32, kind="ExternalInput").ap()
    out = nc.dram_tensor("out", [TSH, OSH], F32, kind="ExternalOutput").ap()

    with tile.TileContext(nc) as tc, ExitStack() as ctx:
        const = ctx.enter_context(tc.tile_pool(name="const", bufs=1))
        wqpool = ctx.enter_context(tc.tile_pool(name="wqp", bufs=NKQ))
        xpool = ctx.enter_context(tc.tile_pool(name="xp", bufs=2))
        k8pool = ctx.enter_context(tc.tile_pool(name="k8p", bufs=3))
        ktpool = ctx.enter_context(tc.tile_pool(name="ktp", bufs=4))
        smalls = ctx.enter_context(tc.tile_pool(name="smalls", bufs=4))
        opool = ctx.enter_context(tc.tile_pool(name="osb", bufs=2))
        psum_m = ctx.enter_context(
            tc.tile_pool(name="psm", bufs=8, space="PSUM"))

        scat = const.tile([128, 2], F32)
        nc.sync.dma_start(out=scat, in_=sca)
        w_scale = scat[:, 1:2]

        # Resident pre-quantized w, 8 chunks of 2 s-planes each.
        wq = [None] * NKQ

        def _ensure_wq(q):
            if wq[q] is None:
                wqt = wqpool.tile([128, 2, 2, OSH], FP8, tag="wq",
                                  name=f"wq{q}")
                nc.sync.dma_start(out=wqt, in_=wts8[:, 2 * q:2 * q + 2, :, :])
                wq[q] = wqt
            return wq[q]

        # Anti-diagonal permutation for reversing per-partition vectors
        # (SwInterleave reverses stationary columns; the host feeds token
        # rows pre-reversed so PSUM comes out ascending, and f crosses the
        # reversal via a tiny R @ f matmul).
        rmat = const.tile([128, 128], F32)
        nc.gpsimd.memset(rmat, 0.0)
        nc.gpsimd.affine_select(
            out=rmat, in_=rmat, compare_op=ALU.not_equal, fill=1.0,
            base=-127, pattern=[[1, 128]], channel_multiplier=1)

        for rep, tt in ((r, t) for r in range(repeat) for t in range(NT)):
            xt = xpool.tile([128, DIN], F32, tag="xt")
            for h in range(2):
                nc.sync.dma_start(
                    out=xt[:, h * 2048:(h + 1) * 2048],
                    in_=xs[tt * 128:(tt + 1) * 128,
                           h * 2048:(h + 1) * 2048])
            if rep == 0 and tt == 0:
                for q in range(NKQ):
                    _ensure_wq(q)
            amax2 = smalls.tile([128, 2], F32, tag="amax2")
            for h in range(2):
                nc.vector.tensor_reduce(
                    out=amax2[:, h:h + 1], in_=xt[:, h * 2048:(h + 1) * 2048],
                    axis=mybir.AxisListType.X, op=ALU.max,
                    apply_absolute_value=True)
            amax = smalls.tile([128, 1], F32, tag="amax")
            nc.vector.tensor_reduce(
                out=amax, in_=amax2, axis=mybir.AxisListType.X, op=ALU.max)
            nc.vector.tensor_scalar_max(amax, amax, EPS)
            s_ap = smalls.tile([128, 1], F32, tag="s_ap")
            nc.vector.reciprocal(out=s_ap, in_=amax)        # 1/amax
            nc.vector.tensor_scalar_mul(s_ap, s_ap, 7.0)    # s = 7/amax
            f_ap = smalls.tile([128, 1], F32, tag="f_ap")
            nc.vector.tensor_scalar(
                out=f_ap, in0=amax, scalar1=1.0 / 7.0, scalar2=w_scale,
                op0=ALU.mult, op1=ALU.mult)                 # scale*amax/7
            # f follows the (reversed) fed row order; PSUM rows come out
            # in token order, so reverse f with the permutation matmul.
            fp = psum_m.tile([128, 1], F32, tag="psm", name=f"fp{tt}")
            nc.tensor.matmul(fp, rmat, f_ap, start=True, stop=True)
            f_rev = smalls.tile([128, 1], F32, tag="f_rev")
            nc.vector.tensor_copy(out=f_rev, in_=fp)
            # y = x*s + MAGIC (in-place; integer part is k+MAGIC) on the
            # otherwise-idle GpSimd; ACT subtracts MAGIC and casts to fp8;
            # the xbar DMA then block-transposes fp8 PAIRS (as uint16):
            # kt[p, s, t] holds (k[t, s*256+2p], k[t, s*256+2p+1]).
            k8 = k8pool.tile([128, DIN], FP8, tag="k8")
            kts = [ktpool.tile([128, 8, 128], mybir.dt.bfloat16, tag="kt",
                               name=f"kt{tt}_{h}") for h in range(2)]
            for h in range(2):
                for ib in range(4):
                    c0 = h * 2048 + ib * 512
                    nc.gpsimd.tensor_scalar(
                        out=xt[:, c0:c0 + 512], in0=xt[:, c0:c0 + 512],
                        scalar1=s_ap, scalar2=MAGIC,
                        op0=ALU.mult, op1=ALU.add)
                nc.scalar.activation(
                    out=k8[:, h * 2048:(h + 1) * 2048],
                    in_=xt[:, h * 2048:(h + 1) * 2048],
                    func=ACTF.Copy, bias=-MAGIC, scale=1.0)
                nc.sync.dma_start(
                    out=kts[h],
                    in_=k8.bitcast(mybir.dt.bfloat16)[:, h * 1024:
                                                      (h + 1) * 1024],
                    transpose=True)
            # DoubleRow fp8 matmuls: lhsT [128,2,128] = fp8-pair view of a
            # kt s-plane (contraction i = s*256+2p+b), rhs [128,2,512] the
            # matching wq slice; accumulate 16 s-planes into each bank.
            # o covered in two half-sweeps of 4 PSUM banks each.
            # Single 16-step sweep over all 8 o-chunks: each wq chunk is
            # consumed once per tile (halves the chunk-gating during the
            # wq fill) and each lhsT load feeds 8 matmuls.
            pss = [psum_m.tile([128, 512], F32, tag="psm",
                               name=f"ps{tt}_{i}")
                   for i in range(8)]
            for s in range(16):
                lhsT = kts[s // 8][:, s % 8, :].bitcast(FP8).rearrange(
                    "p (i m) -> p i m", i=2)
                wqt = wq[s // 2]
                for oc in range(8):
                    nc.tensor.matmul(
                        pss[oc], lhsT,
                        wqt[:, s % 2, :, oc * 512:(oc + 1) * 512],
                        start=(s == 0), stop=(s == 15),
                        perf_mode=mybir.MatmulPerfMode
                        .DoubleRowSwInterleave)
            for half in range(2):
                osb = opool.tile([128, 2048], F32, tag="osb")
                for oc4 in range(4):
                    oc = half * 4 + oc4
                    if oc4 % 2 == 0:
                        nc.scalar.activation(
                            out=osb[:, oc4 * 512:(oc4 + 1) * 512],
                            in_=pss[oc],
                            func=ACTF.Copy, bias=0.0, scale=f_rev)
                    else:
                        nc.vector.tensor_scalar(
                            out=osb[:, oc4 * 512:(oc4 + 1) * 512],
                            in0=pss[oc],
                            scalar1=f_rev, scalar2=None, op0=ALU.mult)
                nc.sync.dma_start(
                    out=out[tt * 128:(tt + 1) * 128,
                            half * 2048:(half + 1) * 2048],
                    in_=osb)
    nc.compile()
    return nc


def _get_ncs():
    if "scale" not in _CACHE:
        _CACHE["scale"] = _build_scale_nc()
    if "wquant" not in _CACHE:
        _CACHE["wquant"] = _build_wquant_nc()
    if "main" not in _CACHE:
        _CACHE["main"] = _build_main_nc()
    return _CACHE["scale"], _CACHE["wquant"], _CACHE["main"]


def kernel(x: np.ndarray, latent_weight: np.ndarray,
           _collect=None) -> np.ndarray:
    x = np.ascontiguousarray(x, dtype=np.float32)
    wT = np.ascontiguousarray(latent_weight.T.astype(np.float32))
    nc_scale, nc_wq, nc_main = _get_ncs()
    core_ids = list(range(NCORES))
    fp8np = mybir.dt.np(FP8)

    segs = [np.ascontiguousarray(wT[c * WSEG:(c + 1) * WSEG, :])
            for c in core_ids]
    in1 = [{"wseg": segs[c]} for c in core_ids]
    r1 = run_bass_kernel_spmd(nc_scale, in1, core_ids=core_ids)
    total = np.float64(0.0)
    for c in core_ids:
        total += r1.results[c]["psums"].astype(np.float64).sum()
    mean = np.float32(total / (DIN * DOUT))
    scale = np.maximum(mean, np.float32(EPS))
    inv_scale = np.float32(1.0) / scale

    sca = np.empty((128, 2), dtype=np.float32)
    sca[:, 0] = inv_scale
    sca[:, 1] = scale
    in2 = [{"wseg": segs[c], "sca": sca} for c in core_ids]
    r2 = run_bass_kernel_spmd(nc_wq, in2, core_ids=core_ids)
    wq_full = np.empty((DIN, DOUT), dtype=fp8np)
    for c in core_ids:
        wq_full[c * WSEG:(c + 1) * WSEG, :] = r2.results[c]["wq8"]

    # Pair-interleaved layout for the fp8-pair DMA transpose convention:
    # wq_dr[p, s, b, o] = wq_full[s*256 + 2p + b, o].
    wq_dr = np.ascontiguousarray(
        wq_full.reshape(16, 128, 2, DOUT).transpose(1, 0, 2, 3))
    in3 = []
    for c in core_ids:
        tg = c // OG
        xsh = x[tg * TSH:(tg + 1) * TSH, :]
        xsh = np.ascontiguousarray(
            xsh.reshape(NT, 128, DIN)[:, ::-1, :].reshape(TSH, DIN))
        in3.append({
            "xs": xsh,
            "wts8": wq_dr,
            "sca": sca,
        })
    r3 = run_bass_kernel_spmd(nc_main, in3, core_ids=core_ids)

    outp = np.empty((TOK, DOUT), dtype=np.float32)
    for c in core_ids:
        tg, og = c // OG, c % OG
        outp[tg * TSH:(tg + 1) * TSH, og * OSH:(og + 1) * OSH] = \
            r3.results[c]["out"]
    if _collect is not None:
        _collect["r1"] = r1
        _collect["r2"] = r2
        _collect["r3"] = r3
    return outp



# revision 23
# speedup vs baseline: 1.1470x; 1.1470x over previous
"""BitLinear (BitNet a4.x-style) Trainium2 kernel.

Computes  out = act_quant_int4(x) @ ste_ternary(w).T  for
x:[8192,4096] f32, w:[4096,4096] f32, on 8 NeuronCores.

Math structure exploited:
  - act_quant_int4(x) rows are  k/s_t  with integer k in [-7,7],
    s_t = 7/amax_t  (per-token).  The clip to [-8,7] is a no-op since
    |x*s| <= 7 by construction.
  - ste_ternary(w) = q * scale with q in {-1,0,1},
    scale = max(mean|w|, 1e-8)  (global scalar).
  - So out[t,o] = (scale * amax_t / 7) * sum_i k[t,i] * q[o,i].
    The inner sum is an exact small-integer dot product computed on the
    PE array with fp8 DoubleRow matmuls (exact fp32 PSUM accumulation);
    rows are scaled by f_t = scale*amax_t/7 during PSUM eviction and
    written out in bf16 (host widens to f32; ~2e-3 rel err, well under
    the 2e-2 gate).

Three launches on 8 cores (an exact global ternary scale is required --
approximate per-shard scales measurably fail the accuracy gate, and
on-device collectives are far too expensive under this fabric):
  1. wscale: per-core partial |w| sums over a 512-row shard of wT in
     128-element chunks; host finishes the reduction in f64.
  2. wquant: ternarize the shard with the exact scale (clamp chain on
     DVE/Pool, magic-constant rounding, fp8 cast on ACT), writing wq8
     directly in the pair-interleaved DRAM layout main consumes.
  3. main, data-parallel over tokens: per 128-token tile: amax
     (DVE+Pool) -> s=7/amax, f=amax*scale/7 (DVE) -> y=x*s+MAGIC (Pool)
     -> fp8 k via ACT bias=-MAGIC -> activation transpose (bf16-bitcast
     PE transposes, bit-preserving, plus DMA-xbar for a couple of tiles
     to balance engines) -> ACT pair-shuffle eviction to kt[c,b,t] ->
     plain-DoubleRow fp8 matmuls (probe-verified [p,b,m] mapping, no
     SwInterleave reversal) accumulating 16 contraction planes into
     rotating PSUM bank pairs per 1024-feature stripe -> DVE eviction
     with *f to bf16 -> store.  Work is emitted in predicted-ready
     order so the PE stays fed while x tiles and wq stripes stream.
"""

import numpy as np
from contextlib import ExitStack

import concourse.bacc as bacc
import concourse.bass as bass
import concourse.mybir as mybir
import concourse.tile as tile
from concourse.bass_utils import run_bass_kernel_spmd

F32 = mybir.dt.float32
FP8 = mybir.dt.float8e4
BF16 = mybir.dt.bfloat16
ALU = mybir.AluOpType
ACTF = mybir.ActivationFunctionType
DR = mybir.MatmulPerfMode.DoubleRow

TOK, DIN, DOUT = 8192, 4096, 4096
NCORES = 8
TSH = TOK // NCORES      # 1024 tokens per core
NT = TSH // 128          # 8 token tiles per core
WSEG = DIN // NCORES     # 512 wT rows per core in the w launches
NPLANES = DIN // 256     # 16 DoubleRow contraction planes
NOC = 8                  # 512-wide feature chunks
NSTRIPE = 4              # 1024-wide feature stripes (2 chunks each)
MAGIC = 12582912.0       # 1.5*2^23: float add/sub round-to-nearest-int
CLAMP = float(np.nextafter(np.float32(1.5), np.float32(0.0)))
EPS = 1e-8

_CACHE = {}


def _build_wscale_nc():
    """Per-core partial |w| sums over the 512-row shard, in 128-element
    chunks (fp32 accumulation error ~1e-7; host finishes in f64)."""
    nc = bacc.Bacc("TRN2", target_bir_lowering=False, debug=False,
                   num_devices=NCORES)
    wseg = nc.dram_tensor("wseg", [WSEG, DIN], F32,
                          kind="ExternalInput").ap()
    psums = nc.dram_tensor("psums", [128, 128], F32,
                           kind="ExternalOutput").ap()
    with tile.TileContext(nc) as tc, ExitStack() as ctx:
        pool = ctx.enter_context(tc.tile_pool(name="w", bufs=4))
        spool = ctx.enter_context(tc.tile_pool(name="s", bufs=1))
        sums = spool.tile([128, 8, 16], F32)
        for i in range(8):
            r0, c0 = (i // 2) * 128, (i % 2) * 2048
            wt = pool.tile([128, 16, 128], F32, tag="wt")
            nc.sync.dma_start(
                out=wt,
                in_=wseg[r0:r0 + 128, c0:c0 + 2048].rearrange(
                    "p (a b) -> p a b", a=16))
            nc.vector.tensor_reduce(
                out=sums[:, i, :], in_=wt, axis=mybir.AxisListType.X,
                op=ALU.add, apply_absolute_value=True)
        nc.sync.dma_start(out=psums, in_=sums.rearrange("p a b -> p (a b)"))
    nc.compile()
    return nc


def _build_wquant_nc():
    """Ternarize the [512, 4096] shard of wT into fp8, writing straight
    into the pair-interleaved layout the main launch consumes:
      wq8[s_local, p, b, o] = q[i, o],  i = c*512 + s_local*256 + 2p + b.
    Shard row r = rb*128 + rr maps to (s_local=rb//2, p=64*(rb%2)+rr//2,
    b=rr%2).  Stores are issued from ACT right after the cast so the SP
    load stream is never blocked."""
    nc = bacc.Bacc("TRN2", target_bir_lowering=False, debug=False,
                   num_devices=NCORES)
    wseg = nc.dram_tensor("wseg", [WSEG, DIN], F32,
                          kind="ExternalInput").ap()
    sca = nc.dram_tensor("sca", [128, 2], F32, kind="ExternalInput").ap()
    wq8 = nc.dram_tensor("wq8", [2, 128, 2, DIN], FP8,
                         kind="ExternalOutput").ap()
    with tile.TileContext(nc) as tc, ExitStack() as ctx:
        const = ctx.enter_context(tc.tile_pool(name="const", bufs=1))
        pool = ctx.enter_context(tc.tile_pool(name="w", bufs=4))
        qpool = ctx.enter_context(tc.tile_pool(name="q", bufs=4))
        scat = const.tile([128, 2], F32)
        nc.sync.dma_start(out=scat, in_=sca)
        for i in range(8):
            rb, h = i // 2, i % 2
            r0, c0 = rb * 128, h * 2048
            wt = pool.tile([128, 2048], F32, tag="wt")
            nc.sync.dma_start(out=wt, in_=wseg[r0:r0 + 128, c0:c0 + 2048])
            nc.vector.tensor_scalar(
                out=wt, in0=wt, scalar1=scat[:, 0:1], scalar2=CLAMP,
                op0=ALU.mult, op1=ALU.min)
            eng2 = nc.gpsimd if i % 2 == 0 else nc.vector
            eng2.tensor_scalar(
                out=wt, in0=wt, scalar1=-CLAMP, scalar2=MAGIC,
                op0=ALU.max, op1=ALU.add)
            qt = qpool.tile([128, 2048], FP8, tag="qt")
            nc.scalar.activation(out=qt, in_=wt, func=ACTF.Copy,
                                 bias=-MAGIC, scale=1.0)
            nc.scalar.dma_start(
                out=wq8[rb // 2, 64 * (rb % 2):64 * (rb % 2) + 64, :,
                        c0:c0 + 2048].rearrange("p b o -> (p b) o"),
                in_=qt)
    nc.compile()
    return nc


def _build_main_nc(n_xbar=0, timing=None, record=None):
    """Main data-parallel launch; see module docstring section 3."""
    nc = bacc.Bacc("TRN2", target_bir_lowering=False, debug=False,
                   num_devices=NCORES)
    xs = nc.dram_tensor("xs", [TSH, DIN], F32, kind="ExternalInput").ap()
    # Pair-interleaved quantized weights: wts8[s, p, b, o] = q[s*256+2p+b, o]
    wts8 = nc.dram_tensor("wts8", [NPLANES, 128, 2, DOUT], FP8,
                          kind="ExternalInput").ap()
    sca = nc.dram_tensor("sca", [128, 1], F32, kind="ExternalInput").ap()
    idt = nc.dram_tensor("idt", [128, 128], BF16, kind="ExternalInput").ap()
    out = nc.dram_tensor("out", [TSH, DOUT], BF16, kind="ExternalOutput").ap()

    with tile.TileContext(nc) as tc, ExitStack() as ctx:
        const = ctx.enter_context(tc.tile_pool(name="const", bufs=1))
        wqpool = ctx.enter_context(tc.tile_pool(name="wqp", bufs=NOC))
        xpool = ctx.enter_context(tc.tile_pool(name="xp", bufs=2))
        k8pool = ctx.enter_context(tc.tile_pool(name="k8p", bufs=2))
        ktpool = ctx.enter_context(tc.tile_pool(name="ktp", bufs=2 * NT))
        smalls = ctx.enter_context(tc.tile_pool(name="smalls", bufs=2 * NT))
        opool = ctx.enter_context(tc.tile_pool(name="osb", bufs=3))
        accpool = ctx.enter_context(
            tc.tile_pool(name="acc", bufs=3, space="PSUM"))
        tpspool = ctx.enter_context(
            tc.tile_pool(name="tps", bufs=2, space="PSUM"))

        scat = const.tile([128, 1], F32)
        nc.sync.dma_start(out=scat, in_=sca)
        ident = const.tile([128, 128], BF16)
        nc.sync.dma_start(out=ident, in_=idt)

        # ---- emission event list, ordered by predicted ready time ----
        events = []   # (time, seq, kind, payload)
        seq = [0]

        def emit(t, kind, payload):
            if timing is not None:
                t = timing.get((kind, payload), t)
            events.append((t, seq[0], kind, payload))
            seq[0] += 1

        XL = 5.825   # x tile load [128, 4096] f32
        OCL = 5.825  # wq oc-chunk load (two b-half DMAs)
        # Load order: x0 oc0 oc1 x1 oc2 x2 oc3 x3 oc4 x4 oc5 x5 oc6 x6 oc7 x7
        load_order = [("x", 0), ("oc", 0), ("oc", 1)]
        nx = 1
        for c in range(2, NOC):
            load_order.append(("x", nx)); nx += 1
            load_order.append(("oc", c))
        while nx < NT:
            load_order.append(("x", nx)); nx += 1
        tdma = 0.0
        t_x = [0.0] * NT
        t_oc = [0.0] * NOC
        for kind, i in load_order:
            if kind == "x":
                tdma += XL
                emit(tdma - XL, "xload", i)
                t_x[i] = tdma
            else:
                tdma += OCL
                emit(tdma - OCL, "ocload", i)
                t_oc[i] = tdma
        t_kt = [0.0] * NT
        for t in range(NT):
            # chain: amax (DVE, 4.3+sem) -> schain (3 small DVE) ->
            # quant halves (Pool) -> cast halves (ACT) -> PE transposes
            # -> shuffles (ACT)
            emit(t_x[t] + 0.02, "amax", t)
            emit(t_x[t] + 5.6, "schain", t)
            emit(t_x[t] + 6.8, "fpool", t)
            emit(t_x[t] + 6.6, "quant_h", (t, 0))
            emit(t_x[t] + 7.0, "quant_h", (t, 1))
            emit(t_x[t] + 8.5, "cast_h", (t, 0))
            emit(t_x[t] + 10.4, "cast_h", (t, 1))
            emit(t_x[t] + 9.5, "tr", (t, 0))
            emit(t_x[t] + 11.6, "tr", (t, 1))
            emit(t_x[t] + 10.9, "shuf", (t, 0))
            emit(t_x[t] + 12.8, "shuf", (t, 1))
            t_kt[t] = t_x[t] + 13.0
        # matmul passes: stripe p of tile t
        passes = []
        for t in range(NT):
            for p in range(NSTRIPE):
                rdy = max(t_kt[t], t_oc[2 * p + 1] + 1.0)
                passes.append((rdy, t, p))
        passes.sort()
        pe_t = 0.0
        for rdy, t, p in passes:
            pe_t = max(pe_t, rdy) + 3.6
            emit(pe_t - 3.6 + 1e-3, "pass", (t, p))
            emit(pe_t + 0.9, "evict", (t, p))

        # Topological fix-up: whatever the (possibly measured) times say,
        # an event may not be emitted before events that create the tile
        # objects it references.
        tmap = {}
        for tt, sq, kind, payload in events:
            tmap[(kind, payload)] = tt

        def bump(key, *prereqs):
            lo = max((tmap[k] for k in prereqs if k in tmap), default=None)
            if lo is not None and tmap[key] <= lo:
                tmap[key] = lo + 1e-4
        for t in range(NT):
            bump(("amax", t), ("xload", t))
            bump(("schain", t), ("amax", t))
            bump(("fpool", t), ("amax", t))
            for h in range(2):
                bump(("quant_h", (t, h)), ("schain", t))
                bump(("cast_h", (t, h)), ("quant_h", (t, h)))
                bump(("tr", (t, h)), ("cast_h", (t, h)))
                bump(("shuf", (t, h)), ("tr", (t, h)))
        for t in range(NT):
            for p in range(NSTRIPE):
                bump(("pass", (t, p)), ("schain", t),
                     ("ocload", 2 * p), ("ocload", 2 * p + 1))
                bump(("evict", (t, p)), ("pass", (t, p)), ("fpool", t))
        events = [(tmap[(kind, payload)], sq, kind, payload)
                  for tt, sq, kind, payload in events]

        # ---- state built during emission ----
        wq = [None] * NOC
        xt = [None] * NT
        k8 = [None] * NT
        kt = [[None, None] for _ in range(NT)]
        f_ap = [None] * NT
        s_ap = [None] * NT
        sm_t = [None] * NT
        acc_tiles = {}
        trbuf = {}
        nev = [0]

        for _, _, kind, payload in sorted(events):
            _n0 = nc.next_id() if record is not None else 0
            if kind == "xload":
                t = payload
                xt[t] = xpool.tile([128, DIN], F32, tag="xt", name=f"xt{t}")
                nc.sync.dma_start(
                    out=xt[t], in_=xs[t * 128:(t + 1) * 128, :])
            elif kind == "ocload":
                c = payload
                wq[c] = wqpool.tile([128, NPLANES, 2, 512], FP8, tag="wq",
                                    name=f"wq{c}")
                for b in range(2):
                    nc.sync.dma_start(
                        out=wq[c][:, :, b, :],
                        in_=wts8.rearrange("s p b o -> p s b o")[
                            :, :, b, c * 512:(c + 1) * 512])
            elif kind == "amax":
                t = payload
                sm = smalls.tile([128, 4], F32, tag="sch", name=f"sch{t}")
                sm_t[t] = sm
                s_ap[t] = sm[:, 2:3]
                f_ap[t] = sm[:, 3:4]
                nc.vector.tensor_reduce(
                    out=sm[:, 0:1], in_=xt[t],
                    axis=mybir.AxisListType.X, op=ALU.max,
                    apply_absolute_value=True)
                for h2 in range(2):
                    kt[t][h2] = ktpool.tile([128, 8, 2, 128], FP8,
                                            tag="kt", name=f"kt{t}_{h2}")
            elif kind == "schain":
                t = payload
                sm = sm_t[t]
                amax = sm[:, 0:1]
                nc.vector.tensor_scalar_mul(sm[:, 1:2], amax, 1.0 / 7.0)
                nc.vector.reciprocal(out=sm[:, 2:3], in_=sm[:, 1:2])
                s_ap[t] = sm[:, 2:3]
            elif kind == "fpool":
                t = payload
                sm = sm_t[t]
                nc.gpsimd.tensor_scalar(
                    out=f_ap[t], in0=sm[:, 0:1], scalar1=scat[:, 0:1],
                    scalar2=None, op0=ALU.mult)
            elif kind == "quant_h":
                t, h = payload
                nc.gpsimd.tensor_scalar(
                    out=xt[t][:, h * 2048:(h + 1) * 2048],
                    in0=xt[t][:, h * 2048:(h + 1) * 2048],
                    scalar1=s_ap[t], scalar2=MAGIC,
                    op0=ALU.mult, op1=ALU.add)
            elif kind == "cast_h":
                t, h = payload
                if h == 0:
                    k8[t] = k8pool.tile([128, DIN], FP8, tag="k8",
                                        name=f"k8_{t}")
                eng = nc.gpsimd if h == 0 else nc.vector
                eng.tensor_scalar(
                    out=k8[t][:, h * 2048:(h + 1) * 2048],
                    in0=xt[t][:, h * 2048:(h + 1) * 2048],
                    scalar1=-MAGIC, scalar2=None, op0=ALU.add)
            elif kind == "tr":
                t, h = payload
                k16 = k8[t].bitcast(BF16)  # [128, 2048] u16-pairs
                tps = tpspool.tile([128, 8, 128], BF16, tag="tps",
                                   name=f"tps{t}_{h}")
                trbuf[(t, h)] = tps
                for gi in range(8):
                    nc.tensor.transpose(
                        tps[:, gi, :],
                        k16[:, h * 1024 + gi * 128:
                            h * 1024 + (gi + 1) * 128],
                        ident)
            elif kind == "shuf":
                t, h = payload
                nc.vector.tensor_copy(
                    out=kt[t][h],
                    in_=trbuf[(t, h)].bitcast(FP8).rearrange(
                        "p g (t b) -> p g b t", b=2))
            elif kind == "pass":
                t, p = payload
                acc = accpool.tile([128, 1024], F32, tag="acc",
                                   name=f"acc{t}_{p}")
                acc_tiles[(t, p)] = acc
                for st in range(NPLANES):
                    lhsT = kt[t][st // 8][:, st % 8, :, :]
                    for j in range(2):
                        nc.tensor.matmul(
                            acc[:, j * 512:(j + 1) * 512], lhsT,
                            wq[2 * p + j][:, st, :, :],
                            start=(st == 0), stop=(st == NPLANES - 1),
                            perf_mode=DR)
            elif kind == "evict":
                t, p = payload
                acc = acc_tiles[(t, p)]
                ot = opool.tile([128, 1024], BF16, tag="osb",
                                name=f"osb{t}_{p}")
                nc.scalar.activation(
                    out=ot, in_=acc, func=ACTF.Copy, bias=0.0,
                    scale=f_ap[t])
                nc.scalar.dma_start(
                    out=out[t * 128:(t + 1) * 128,
                            p * 1024:(p + 1) * 1024],
                    in_=ot)
            if record is not None:
                record.append((kind, payload,
                               [f"I-{i}" for i in range(_n0 + 1,
                                                        nc.next_id())]))

    nc.compile()
    return nc


def _get_ncs():
    if "wscale" not in _CACHE:
        _CACHE["wscale"] = _build_wscale_nc()
    if "wquant" not in _CACHE:
        _CACHE["wquant"] = _build_wquant_nc()
    if "main" not in _CACHE:
        _CACHE["main"] = _build_main_nc()
    return _CACHE["wscale"], _CACHE["wquant"], _CACHE["main"]


def kernel(x: np.ndarray, latent_weight: np.ndarray,
           _collect=None) -> np.ndarray:
    x = np.ascontiguousarray(x, dtype=np.float32)
    wT = np.ascontiguousarray(latent_weight.T.astype(np.float32))
    nc_scale, nc_wq, nc_main = _get_ncs()
    core_ids = list(range(NCORES))
    fp8np = mybir.dt.np(FP8)
    bf16np = mybir.dt.np(BF16)

    segs = [wT[c * WSEG:(c + 1) * WSEG, :] for c in core_ids]
    in1 = [{"wseg": segs[c]} for c in core_ids]
    r1 = run_bass_kernel_spmd(nc_scale, in1, core_ids=core_ids)
    total = np.float64(0.0)
    for c in core_ids:
        total += r1.results[c]["psums"].astype(np.float64).sum()
    mean = np.float32(total / (DIN * DOUT))
    scale = np.maximum(mean, np.float32(EPS))
    inv_scale = np.float32(1.0) / scale

    sca2 = np.empty((128, 2), dtype=np.float32)
    sca2[:, 0] = inv_scale
    sca2[:, 1] = scale
    in2 = [{"wseg": segs[c], "sca": sca2} for c in core_ids]
    r2 = run_bass_kernel_spmd(nc_wq, in2, core_ids=core_ids)
    # wts8[s, p, b, o] with s = 2c + s_local: concat per-core outputs
    wts8 = np.ascontiguousarray(
        np.concatenate([r2.results[c]["wq8"] for c in core_ids], axis=0))

    sca = np.full((128, 1), scale / np.float32(7.0), dtype=np.float32)
    idt = np.eye(128, dtype=np.float32).astype(bf16np)
    in3 = [{"xs": x[c * TSH:(c + 1) * TSH, :], "wts8": wts8, "sca": sca,
            "idt": idt} for c in core_ids]
    r3 = run_bass_kernel_spmd(nc_main, in3, core_ids=core_ids)

    outp = np.empty((TOK, DOUT), dtype=np.float32)
    for c in core_ids:
        outp[c * TSH:(c + 1) * TSH, :] = \
            r3.results[c]["out"].astype(np.float32)
    if _collect is not None:
        _collect["r1"] = r1
        _collect["r2"] = r2
        _collect["r3"] = r3
    return outp
